# revision 8
# baseline (speedup 1.0000x reference)
"""Trainium2 Bass kernel: hexagram-routed MoE (dense top-2 of 6 experts).

Strategy: data-parallel over tokens — 8192 tokens sharded 1024/core over 8
NeuronCores, router + expert weights replicated. Everything on-chip is
feature-major (features on partitions, tokens on the free dim) so no input
transposes are needed beyond the host-side x.T per shard. Expert GEMMs run
with float32r tensor-engine dtype (full-rate rows at N=512); the router and
top-2 selection stay exact fp32.
"""

import sys

sys.path.insert(0, "/opt/trn_rl_repo")

import numpy as np

import concourse.bass as bass
import concourse.mybir as mybir
import concourse.tile as tile
from concourse import bacc
from concourse.bass_utils import run_bass_kernel_spmd
from concourse.masks import make_identity

P = 128
B, T, D = 4, 2048, 1024
E, H = 6, 512
N_CORES = 8
N = (B * T) // N_CORES  # tokens per core
KD = D // P  # 8 k-tiles over d_model
KH = H // P  # 4 k-tiles over d_expert
MD = D // P  # 8 m-tiles over d_model output
MH = H // P  # 4 m-tiles over d_expert output
NH = 2  # halves of the 1024-token shard (psum free dim 512)
NSUB = N // NH
QUANT_TEMP = 0.3

F32 = mybir.dt.float32
F32R = mybir.dt.float32r
GELU = mybir.ActivationFunctionType.Gelu_apprx_tanh
TANH = mybir.ActivationFunctionType.Tanh
SIGM = mybir.ActivationFunctionType.Sigmoid
COPY = mybir.ActivationFunctionType.Copy

_CACHED_NC = None


def _r(ap):
    return ap.bitcast(F32R)


def _f(ap):
    return ap.bitcast(F32)


def build_nc():
    nc = bacc.Bacc("TRN2", target_bir_lowering=False, debug=False,
                   num_devices=N_CORES)
    xT = nc.dram_tensor("xT", [D, N], F32R, kind="ExternalInput")
    w1f = nc.dram_tensor("w1f", [D, E * H], F32R, kind="ExternalInput")
    w2f = nc.dram_tensor("w2f", [E * H, D], F32R, kind="ExternalInput")
    pwT = nc.dram_tensor("pwT", [D, 6], F32, kind="ExternalInput")
    anchT = nc.dram_tensor("anchT", [6, 6], F32, kind="ExternalInput")
    spos = nc.dram_tensor("spos", [P, 1], F32, kind="ExternalInput")
    sneg = nc.dram_tensor("sneg", [P, 1], F32, kind="ExternalInput")
    selm = nc.dram_tensor("selm", [6, E * P], F32, kind="ExternalInput")
    yT = nc.dram_tensor("yT", [D, N], F32, kind="ExternalOutput")

    xT3 = xT.rearrange("(a p) n -> a p n", p=P)        # [KD, 128, N]
    w1f3 = w1f.rearrange("(a p) h -> a p h", p=P)      # [KD, 128, E*H]
    w2f3 = w2f.rearrange("(a p) d -> a p d", p=P)      # [E*KH, 128, D]
    pwT3 = pwT.rearrange("(a p) e -> a p e", p=P)      # [KD, 128, 6]
    yT3 = yT.rearrange("(a p) n -> a p n", p=P)        # [MD, 128, N]

    with tile.TileContext(nc) as tc:
        with (
            tc.tile_pool(name="xt", bufs=1) as xt_pool,
            tc.tile_pool(name="const", bufs=1) as cpool,
            tc.tile_pool(name="router", bufs=1) as rpool,
            tc.tile_pool(name="w1p", bufs=2) as w1p,
            tc.tile_pool(name="w2p", bufs=2) as w2p,
            tc.tile_pool(name="hp", bufs=2) as hpool,
            tc.tile_pool(name="yp", bufs=1) as ypool,
            tc.tile_pool(name="ps", bufs=2, space="PSUM") as ps,
            tc.tile_pool(name="psh", bufs=4, space="PSUM") as psh,
            tc.tile_pool(name="psy", bufs=2, space="PSUM") as psy,
        ):
            # ---- resident inputs ----
            xt = [xt_pool.tile([P, N], F32R, tag=f"xt{k}", name=f"xt{k}")
                  for k in range(KD)]
            for k in range(KD):
                nc.sync.dma_start(xt[k][:], xT3[k])

            pw = cpool.tile([P, KD, 6], F32)
            nc.sync.dma_start(pw[:], pwT3.rearrange("a p e -> p a e"))
            anch = cpool.tile([6, 6], F32)
            nc.sync.dma_start(anch[:], anchT[:])
            sp = cpool.tile([P, 1], F32)
            nc.sync.dma_start(sp[:], spos[:])
            sn = cpool.tile([P, 1], F32)
            nc.sync.dma_start(sn[:], sneg[:])
            ident = cpool.tile([P, P], F32)
            make_identity(nc, ident[:])
            # sel[:, e*128:(e+1)*128] has row e all-ones: sel_e.T @ ew
            # broadcasts expert row e across all 128 partitions
            sel = cpool.tile([6, E * P], F32)
            nc.sync.dma_start(sel[:], selm[:])

            # ---- router ----
            q = rpool.tile([6, N], F32)
            dot = rpool.tile([6, N], F32)
            dot_t = rpool.tile([P, KD, 6], F32)
            ew_t = rpool.tile([P, KD, 6], F32)
            ew = rpool.tile([6, N], F32)
            ew_b = rpool.tile([P, E, N], F32R)

            for nh in range(NH):
                ns = slice(nh * NSUB, (nh + 1) * NSUB)
                zps = ps.tile([6, NSUB], F32, tag="rps")
                for k in range(KD):
                    nc.tensor.matmul(zps[:], pw[:, k, :], _f(xt[k][:, ns]),
                                     start=(k == 0), stop=(k == KD - 1))
                # q = tanh(z / 0.3)
                nc.scalar.activation(q[:, ns], zps[:], TANH,
                                     scale=1.0 / QUANT_TEMP)
            for nh in range(NH):
                ns = slice(nh * NSUB, (nh + 1) * NSUB)
                dps = ps.tile([6, NSUB], F32, tag="rps")
                nc.tensor.matmul(dps[:], anch[:], q[0:6, ns],
                                 start=True, stop=True)
                nc.scalar.activation(dot[:, ns], dps[:], COPY)

            # token-major tiles for top-2 (PE transpose, 128 tokens each)
            for a in range(KD):
                tps = ps.tile([P, 6], F32, tag="rps")
                nc.tensor.transpose(tps[:], dot[0:6, a * P:(a + 1) * P],
                                    ident[0:6, 0:6])
                nc.scalar.activation(dot_t[:, a, :], tps[:], COPY)

            m1 = rpool.tile([P, KD], F32)
            m2 = rpool.tile([P, KD], F32)
            sig1 = rpool.tile([P, KD], F32)
            sig2 = rpool.tile([P, KD], F32)
            scr = rpool.tile([P, KD, 6], F32)
            for a in range(KD):
                dslc = dot_t[:, a, :]
                nc.vector.reduce_max(m1[:, a:a + 1], dslc,
                                     axis=mybir.AxisListType.X)
                # mask out argmax -> second max
                eq1 = scr[:, a, :]
                nc.vector.tensor_scalar(eq1, dslc, m1[:, a:a + 1], None,
                                        op0=mybir.AluOpType.is_equal)
                nc.vector.tensor_scalar(eq1, eq1, 1e30, None,
                                        op0=mybir.AluOpType.mult)
                nc.vector.tensor_sub(eq1, dslc, eq1)
                nc.vector.reduce_max(m2[:, a:a + 1], eq1,
                                     axis=mybir.AxisListType.X)
                # softmax over the two: w1 = sigmoid((d1-d2)*0.5/ct)
                d12 = sig1[:, a:a + 1]
                nc.vector.tensor_sub(d12, m1[:, a:a + 1], m2[:, a:a + 1])
                nc.scalar.activation(sig2[:, a:a + 1], d12, SIGM, scale=sn[:])
                nc.scalar.activation(sig1[:, a:a + 1], d12, SIGM, scale=sp[:])
            for a in range(KD):
                dslc = dot_t[:, a, :]
                eq1 = scr[:, a, :]
                ewslc = ew_t[:, a, :]
                # ew = (dot==m1)*sig1 + (masked==m2 ... via dot==m2)*sig2
                nc.vector.tensor_scalar(eq1, dslc, m1[:, a:a + 1], None,
                                        op0=mybir.AluOpType.is_equal)
                nc.vector.tensor_scalar(eq1, eq1, sig1[:, a:a + 1], None,
                                        op0=mybir.AluOpType.mult)
                nc.vector.tensor_scalar(ewslc, dslc, m2[:, a:a + 1], None,
                                        op0=mybir.AluOpType.is_equal)
                nc.vector.tensor_scalar(ewslc, ewslc, sig2[:, a:a + 1], None,
                                        op0=mybir.AluOpType.mult)
                nc.vector.tensor_add(ewslc, ewslc, eq1)
            # back to expert-major [6, N]
            for a in range(KD):
                eps = ps.tile([6, P], F32, tag="rps")
                nc.tensor.transpose(eps[0:6, :], ew_t[:, a, :], ident[:, :])
                nc.scalar.activation(ew[0:6, a * P:(a + 1) * P], eps[0:6, :],
                                     COPY)
            # broadcast each expert row across 128 partitions (K=1 matmul)
            for e in range(E):
                for nh in range(NH):
                    ns = slice(nh * NSUB, (nh + 1) * NSUB)
                    bps = ps.tile([P, NSUB], F32, tag="rps")
                    nc.tensor.matmul(bps[:], sel[:, e * P:(e + 1) * P],
                                     ew[0:6, ns], start=True, stop=True)
                    nc.scalar.activation(ew_b[:, e, ns], bps[:], COPY)

            # ---- experts ----
            y_sb = [ypool.tile([P, MD, NSUB], F32, tag=f"ysb{nh}",
                              name=f"ysb{nh}") for nh in range(NH)]
            for e in range(E):
                w1s = w1p.tile([P, KD, H], F32R, tag="w1s")
                nc.sync.dma_start(
                    w1s[:], w1f3[:, :, e * H:(e + 1) * H]
                    .rearrange("a p h -> p a h"))
                w2s = w2p.tile([P, KH, D], F32R, tag="w2s")
                nc.sync.dma_start(
                    w2s[:], w2f3[e * KH:(e + 1) * KH]
                    .rearrange("a p d -> p a d"))
                for nh in range(NH):
                    ns = slice(nh * NSUB, (nh + 1) * NSUB)
                    h_sb = hpool.tile([P, KH, NSUB], F32R, tag="hsb")
                    for hm in range(MH):
                        hps = psh.tile([P, NSUB], F32, tag="hps")
                        for k in range(KD):
                            nc.tensor.matmul(
                                hps[:],
                                w1s[:, k, hm * P:(hm + 1) * P],
                                xt[k][:, ns],
                                start=(k == 0), stop=(k == KD - 1))
                        nc.scalar.activation(h_sb[:, hm, :], hps[:], GELU)
                        nc.vector.tensor_mul(h_sb[:, hm, :], h_sb[:, hm, :],
                                             ew_b[:, e, ns])
                    for m in range(MD):
                        yps = psy.tile([P, NSUB], F32, tag="yps")
                        for hk in range(KH):
                            nc.tensor.matmul(
                                yps[:],
                                w2s[:, hk, m * P:(m + 1) * P],
                                h_sb[:, hk, :],
                                start=(hk == 0), stop=(hk == KH - 1))
                        if e == 0:
                            nc.scalar.activation(y_sb[nh][:, m, :], yps[:],
                                                 COPY)
                        else:
                            nc.vector.tensor_add(y_sb[nh][:, m, :],
                                                 y_sb[nh][:, m, :], yps[:])

            for nh in range(NH):
                ns = slice(nh * NSUB, (nh + 1) * NSUB)
                nc.sync.dma_start(
                    yT3[:, :, ns].rearrange("a p n -> p a n"), y_sb[nh][:])
    nc.compile()
    return nc


def kernel(x, proj_w, anchors, routing_temp, w1, w2):
    global _CACHED_NC
    if _CACHED_NC is None:
        _CACHED_NC = build_nc()
    nc = _CACHED_NC

    xf = np.ascontiguousarray(np.asarray(x, np.float32).reshape(B * T, D))
    w1f = np.ascontiguousarray(
        np.asarray(w1, np.float32).transpose(1, 0, 2).reshape(D, E * H))
    w2f = np.ascontiguousarray(np.asarray(w2, np.float32).reshape(E * H, D))
    pwT = np.ascontiguousarray(np.asarray(proj_w, np.float32).T)
    anchT = np.ascontiguousarray(np.asarray(anchors, np.float32).T)
    ct = max(float(np.asarray(routing_temp, np.float32)), 0.1)
    spos = np.full((P, 1), 0.5 / ct, np.float32)
    sneg = np.full((P, 1), -0.5 / ct, np.float32)
    selm = np.zeros((E, E * P), np.float32)
    for e in range(E):
        selm[e, e * P:(e + 1) * P] = 1.0

    in_maps = []
    for i in range(N_CORES):
        xT = np.ascontiguousarray(xf[i * N:(i + 1) * N].T)
        in_maps.append({"xT": xT, "w1f": w1f, "w2f": w2f, "pwT": pwT,
                        "anchT": anchT, "spos": spos, "sneg": sneg,
                        "selm": selm})

    res = run_bass_kernel_spmd(nc, in_maps, core_ids=list(range(N_CORES)))
    out = np.concatenate([r["yT"].T for r in res.results], axis=0)
    return out.reshape(B, T, D).astype(np.float32)
